# revision 3
# baseline (speedup 1.0000x reference)
"""Builder + host prep for the sparse decoding-attention TRN2 kernel.

Sharding: 8 cores, one KV head per core (tensor parallel over Hkv=8).
Each core computes out[:, h*G:(h+1)*G, :] and sel[:, h*G:(h+1)*G, :]
for its head h; the host concatenates.
"""

import sys as _sys
for _p in ("/opt/trn_rl_repo",):
    if _p not in _sys.path:
        _sys.path.insert(0, _p)


import numpy as np
from ml_dtypes import bfloat16

import concourse.bass as bass
import concourse.bacc as bacc
import concourse.mybir as mybir
from concourse.tile import TileContext

F32 = mybir.dt.float32
BF16 = mybir.dt.bfloat16
I32 = mybir.dt.int32
U32 = mybir.dt.uint32
U16 = mybir.dt.uint16
I16 = mybir.dt.int16

B, H, HKV, D = 4, 32, 8, 128
G = H // HKV          # 4
P, S = 128, 64        # pages, tokens/page
T = P * S             # 8192
KP = 16               # selected pages (15 top + last)
NTILE = 512           # tokens per score matmul tile
NT = T // NTILE       # 16
NEG = -1.0e9
PF = T + 128          # p grid: 8192 cache tokens + current-token slots (pad)
NCH = PF // 128       # 65 transpose/PV chunks
VROWS = T + 128       # v rows per b: tokens + v_hi/v_mid/v_lo x2 + zero pad


def bf16_split3(x32: np.ndarray):
    """Exact 3-term bf16 decomposition of float32 (hi+mid+lo == x bitwise)."""
    x32 = np.asarray(x32, np.float32)
    hi = x32.astype(bfloat16)
    r1 = x32 - hi.astype(np.float32)
    mid = r1.astype(bfloat16)
    r2 = r1 - mid.astype(np.float32)
    lo = r2.astype(bfloat16)
    return hi, mid, lo


def host_prep(q, k, v, kv_scale_quant_orig, k_cache, v_cache,
              lengths_per_sample, timestep):
    """Returns (per_core_inmaps, meta). Uses jax-on-CPU for rope + current
    scores so the math matches the reference bit-for-bit."""
    import jax
    import jax.numpy as jnp

    cpu = jax.devices("cpu")[0]
    pos = float(int(timestep) - 1)
    sm_scale = 1.0 / np.sqrt(D)

    with jax.default_device(cpu):
        def rope_neox(x, p, base=10000.0):
            half = D // 2
            inv_freq = 1.0 / (base ** (jnp.arange(half, dtype=jnp.float32) * (2.0 / D)))
            ang = p * inv_freq
            c, s = jnp.cos(ang), jnp.sin(ang)
            x1, x2 = x[..., :half], x[..., half:]
            return jnp.concatenate([x1 * c - x2 * s, x1 * s + x2 * c], axis=-1)

        qj = jnp.asarray(np.asarray(q, np.float32))
        kj = jnp.asarray(np.asarray(k, np.float32))
        qh = rope_neox(qj, pos).reshape(B, HKV, G, D)
        k_cur = rope_neox(kj, pos)
        cur = jnp.einsum('bhgd,bhd->bhg', qh, k_cur) * sm_scale
        qh = np.asarray(qh)          # [B, HKV, G, D] f32
        cur = np.asarray(cur)        # [B, HKV, G] f32

    kv_scale = np.asarray(kv_scale_quant_orig, np.float32)
    sk, sv = float(kv_scale[0]), float(kv_scale[1])
    scale_k = float(np.float32(sk) * np.float32(sm_scale))

    k_cache = np.asarray(k_cache)
    v_cache = np.asarray(v_cache)
    # K^T per head: [HKV, D, B*T] bf16 (ints 0..127, exact)
    kc_all = np.ascontiguousarray(
        k_cache.transpose(3, 4, 0, 1, 2).astype(bfloat16)).reshape(HKV, D, B * T)
    # V rows per (head, b): [HKV, B, VROWS, D] bf16
    vt_all = np.zeros((HKV, B, VROWS, D), bfloat16)
    vt_all[:, :, :T, :] = v_cache.transpose(3, 0, 1, 2, 4).astype(bfloat16).reshape(
        HKV, B, T, D)
    # the reference appends the RAW current v (no sv dequant); our kernel
    # scales the whole PV output by sv, so store v/sv here (exact 3-term
    # bf16 split of the f32 quotient). Rows T..T+2 pair with p_hi slots,
    # rows T+3..T+5 with p_lo slots.
    v32 = np.asarray(v, np.float32)
    vq = (v32 / np.float32(sv)).astype(np.float32)
    v_hi, v_mid, v_lo = bf16_split3(vq)
    for h in range(HKV):
        for b in range(B):
            for rep in (0, 3):
                vt_all[h, b, T + rep, :] = v_hi[b, h]
                vt_all[h, b, T + rep + 1, :] = v_mid[b, h]
                vt_all[h, b, T + rep + 2, :] = v_lo[b, h]

    lengths = np.maximum(np.asarray(lengths_per_sample, np.int64), 1)
    lengths = np.minimum(lengths, T).astype(np.int64)

    negbuf = np.full((T,), NEG, np.float32)

    per_core = []
    for h in range(HKV):
        hi, mid, lo = bf16_split3(qh[:, h])          # each [B, G, D]
        # q3[d, b*12 + j*4 + g]
        q3 = np.ascontiguousarray(
            np.stack([hi, mid, lo], axis=1)          # [B, 3, G, D]
            .transpose(3, 0, 1, 2)                   # [D, B, 3, G]
            .reshape(D, B * 12))
        curs = np.zeros((128, 1), np.float32)
        for b in range(B):
            for g in range(G):
                curs[32 * b + g, 0] = cur[b, h, g]
        im = {
            "kt": kc_all[h],                         # [128, 32768] bf16
            "q3": q3,                                # [128, 48] bf16
            "curs": curs,                            # [128, 1] f32
            "negbuf": negbuf,                        # [8192] f32
        }
        for b in range(B):
            im[f"vt{b}"] = vt_all[h, b]              # [VROWS, 128] bf16
        per_core.append(im)

    meta = {
        "lengths": [int(x) for x in lengths],
        "scale_k": scale_k,
        "sv": float(np.float32(sv)),
    }
    return per_core, meta


def build_nc(meta):
    """Builds the compiled per-core Bass program (same NEFF for all cores)."""
    lengths = meta["lengths"]
    scale_k = meta["scale_k"]
    sv = meta["sv"]

    nc = bacc.Bacc("TRN2", target_bir_lowering=False, debug=False)

    kt = nc.dram_tensor("kt", [D, B * T], BF16, kind="ExternalInput").ap()
    q3 = nc.dram_tensor("q3", [D, B * 12], BF16, kind="ExternalInput").ap()
    curs = nc.dram_tensor("curs", [128, 1], F32, kind="ExternalInput").ap()
    negbuf = nc.dram_tensor("negbuf", [T], F32, kind="ExternalInput").ap()
    vts = [nc.dram_tensor(f"vt{b}", [VROWS, D], BF16, kind="ExternalInput").ap()
           for b in range(B)]

    o_out = nc.dram_tensor("o_out", [B, G * D], F32, kind="ExternalOutput").ap()
    sel_out = nc.dram_tensor("sel_out", [B, G, KP], I32, kind="ExternalOutput").ap()

    nt_b = [min(NT, (lengths[b] + NTILE - 1) // NTILE) for b in range(B)]
    nt_max = max(nt_b)

    with TileContext(nc) as tc:
        with (
            tc.tile_pool(name="const", bufs=1) as cpool,
            tc.tile_pool(name="scores", bufs=1) as spool,
            tc.tile_pool(name="kt", bufs=4) as kpool,
            tc.tile_pool(name="psum_s", bufs=1, space="PSUM") as pspool,
            tc.tile_pool(name="psum_pv", bufs=1, space="PSUM") as pvpool,
            tc.tile_pool(name="work", bufs=1) as wpool,
            tc.tile_pool(name="vsel", bufs=1) as vpool,
        ):
            q3_sb = cpool.tile([D, B * 12], BF16, tag="q3")
            nc.sync.dma_start(out=q3_sb[:], in_=q3)

            scores_sb = spool.tile([128, T], F32, tag="scores")

            # Persistent rotating PSUM tiles, zeroed once: partitions outside
            # the 4 matmul column groups stay 0.0 so no NaN garbage reaches
            # the stats/top-k path.
            NPS = 4
            psum_tiles = [pspool.tile([128, NTILE], F32, tag=f"ps{i}",
                                      name=f"ps{i}")
                          for i in range(NPS)]
            for pt in psum_tiles:
                nc.vector.memset(pt[:], 0.0)

            # ---- phase 1: scores = (q . k) * scale_k for all tokens ----
            for t in range(nt_max):
                psum = psum_tiles[t % NPS]
                for b in range(B):
                    if t >= nt_b[b]:
                        continue
                    ktile = kpool.tile([D, NTILE], BF16, tag="kt")
                    nc.sync.dma_start(
                        out=ktile[:],
                        in_=kt[:, b * T + t * NTILE: b * T + (t + 1) * NTILE])
                    for j in range(3):
                        nc.tensor.matmul(
                            out=psum[32 * b: 32 * b + G, :],
                            lhsT=q3_sb[:, b * 12 + j * 4: b * 12 + j * 4 + 4],
                            rhs=ktile[:],
                            start=(j == 0),
                            stop=(j == 2),
                            tile_position=(0, 32 * b),
                        )
                nc.scalar.mul(
                    out=scores_sb[:, t * NTILE: (t + 1) * NTILE],
                    in_=psum[:], mul=scale_k)

            # columns past the longest computed tile are never written by the
            # PSUM copies; zero them so the stats reduce reads defined data
            if nt_max < NT:
                nc.vector.memset(scores_sb[:, nt_max * NTILE: T], 0.0)

            # mask the invalid tail [L_b, T) with NEG via broadcast DMA
            for b in range(B):
                L = lengths[b]
                if L < T:
                    nc.sync.dma_start(
                        out=scores_sb[32 * b: 32 * b + G, L:T],
                        in_=negbuf[0: T - L].unsqueeze(0).to_broadcast([G, T - L]))

            # ---- phase 2: page stats + top-k ----
            stats = wpool.tile([128, P], F32, tag="stats")
            nc.vector.reduce_max(
                out=stats[:],
                in_=scores_sb[:].rearrange("p (a b) -> p a b", b=S),
                axis=mybir.AxisListType.X)

            mx_v = wpool.tile([128, 8], F32, tag="mx_v")
            mx_i = wpool.tile([128, 8], U32, tag="mx_i")
            stats2 = wpool.tile([128, P - 1], F32, tag="stats2")
            mx2_v = wpool.tile([128, 8], F32, tag="mx2_v")
            mx2_i = wpool.tile([128, 8], U32, tag="mx2_i")

            nc.vector.max(out=mx_v[:], in_=stats[:, 0: P - 1])
            nc.vector.max_index(out=mx_i[:], in_max=mx_v[:], in_values=stats[:, 0: P - 1])
            nc.vector.match_replace(
                out=stats2[:], in_to_replace=mx_v[:], in_values=stats[:, 0: P - 1],
                imm_value=-1.0e30)
            nc.vector.max(out=mx2_v[:], in_=stats2[:])
            nc.vector.max_index(out=mx2_i[:], in_max=mx2_v[:], in_values=stats2[:])

            sel_i32 = wpool.tile([128, KP], I32, tag="sel_i32")
            nc.vector.tensor_copy(out=sel_i32[:, 0:8], in_=mx_i[:])
            nc.vector.tensor_copy(out=sel_i32[:, 8:15], in_=mx2_i[:, 0:7])
            nc.vector.memset(sel_i32[:, 15:16], P - 1)
            for b in range(B):
                nc.sync.dma_start(out=sel_out[b], in_=sel_i32[32 * b: 32 * b + G, :])

            # ---- phase 3: selection mask + dense masked softmax ----
            # selection set == {stats >= 15th-largest} + forced last page
            thresh = mx2_v[:, 6:7]
            mask01 = wpool.tile([128, P], F32, tag="mask01")
            nc.vector.tensor_scalar(
                out=mask01[:], in0=stats[:], scalar1=thresh, scalar2=None,
                op0=mybir.AluOpType.is_ge)
            nc.vector.memset(mask01[:, P - 1: P], 1.0)
            tmask = wpool.tile([128, P], F32, tag="tmask")
            nc.vector.tensor_scalar(
                out=tmask[:], in0=mask01[:], scalar1=1.0, scalar2=-NEG,
                op0=mybir.AluOpType.subtract, op1=mybir.AluOpType.mult)
            # scores += tmask (0 for selected pages, -1e9 for the rest)
            nc.vector.scalar_tensor_tensor(
                out=scores_sb[:].rearrange("p (a b) -> p a b", b=S),
                in0=tmask[:].unsqueeze(2).to_broadcast([128, P, S]),
                scalar=0.0,
                in1=scores_sb[:].rearrange("p (a b) -> p a b", b=S),
                op0=mybir.AluOpType.add,
                op1=mybir.AluOpType.add)

            curs_sb = wpool.tile([128, 1], F32, tag="curs_sb")
            nc.sync.dma_start(out=curs_sb[:], in_=curs)
            m0 = wpool.tile([128, 1], F32, tag="m0")
            m = wpool.tile([128, 1], F32, tag="m")
            negm = wpool.tile([128, 1], F32, tag="negm")
            sume = wpool.tile([128, 1], F32, tag="sume")
            ecur = wpool.tile([128, 1], F32, tag="ecur")
            sumall = wpool.tile([128, 1], F32, tag="sumall")
            rec = wpool.tile([128, 1], F32, tag="rec")
            nc.vector.reduce_max(out=m0[:], in_=scores_sb[:], axis=mybir.AxisListType.X)
            nc.vector.tensor_tensor(out=m[:], in0=m0[:], in1=curs_sb[:],
                                    op=mybir.AluOpType.max)
            nc.vector.tensor_scalar_mul(negm[:], m[:], -1.0)
            # exp in place over the score grid, with fused running sum
            nc.scalar.activation(
                out=scores_sb[:], in_=scores_sb[:],
                func=mybir.ActivationFunctionType.Exp,
                bias=negm[:], scale=1.0, accum_out=sume[:])
            nc.scalar.activation(
                out=ecur[:], in_=curs_sb[:],
                func=mybir.ActivationFunctionType.Exp,
                bias=negm[:], scale=1.0)
            nc.vector.tensor_tensor(out=sumall[:], in0=sume[:], in1=ecur[:],
                                    op=mybir.AluOpType.add)
            nc.vector.reciprocal(out=rec[:], in_=sumall[:])

            p_full = wpool.tile([128, PF], BF16, tag="p_full")
            nc.vector.tensor_scalar_mul(p_full[:, 0:T], scores_sb[:], rec[:])
            # current-token prob, exact via bf16 hi+lo split:
            # p slots [T..T+2] = p_hi (x v_hi/v_mid/v_lo rows),
            # p slots [T+3..T+5] = p_lo.
            pcur_f = wpool.tile([128, 1], F32, tag="pcur_f")
            phi_f = wpool.tile([128, 1], F32, tag="phi_f")
            plo_f = wpool.tile([128, 1], F32, tag="plo_f")
            nc.vector.tensor_tensor(out=pcur_f[:], in0=ecur[:], in1=rec[:],
                                    op=mybir.AluOpType.mult)
            nc.vector.tensor_copy(out=p_full[:, T: T + 1], in_=pcur_f[:])
            nc.vector.tensor_copy(out=phi_f[:], in_=p_full[:, T: T + 1])
            nc.vector.tensor_tensor(out=plo_f[:], in0=pcur_f[:], in1=phi_f[:],
                                    op=mybir.AluOpType.subtract)
            nc.vector.tensor_copy(out=p_full[:, T + 1: T + 2], in_=p_full[:, T: T + 1])
            nc.vector.tensor_copy(out=p_full[:, T + 2: T + 3], in_=p_full[:, T: T + 1])
            nc.vector.tensor_copy(out=p_full[:, T + 3: T + 4], in_=plo_f[:])
            nc.vector.tensor_copy(out=p_full[:, T + 4: T + 5], in_=p_full[:, T + 3: T + 4])
            nc.vector.tensor_copy(out=p_full[:, T + 5: T + 6], in_=p_full[:, T + 3: T + 4])
            nc.vector.memset(p_full[:, T + 6: PF], 0.0)

            # ---- phase 4: transpose p into [token, row] chunks ----
            pT = wpool.tile([128, NCH, 128], BF16, tag="pT")
            nv_b = [min(64, (lengths[b] + 127) // 128) for b in range(B)]
            nv_max = max(nv_b)
            teng = [nc.sync, nc.scalar]
            tn = 0
            for c in list(range(nv_max)) + [NCH - 1]:
                teng[tn % 2].dma_start_transpose(
                    out=pT[:, c, :], in_=p_full[:, c * 128: (c + 1) * 128])
                tn += 1

            # ---- phase 5: dense PV (prefetched V) + current token ----
            # V chunks live in a persistent SBUF buffer, DMA'd without any
            # dependency on the selection so the loads overlap phase 1.
            nvsum = sum(nv_b) + B
            v_all = vpool.tile([128, nvsum, 128], BF16, tag="v_all")
            vbase = []
            off = 0
            for b in range(B):
                vbase.append(off)
                for c in range(nv_b[b]):
                    nc.sync.dma_start(
                        out=v_all[:, off + c, :],
                        in_=vts[b][c * 128: (c + 1) * 128, :].rearrange(
                            "(o p) d -> p o d", p=128))
                nc.sync.dma_start(
                    out=v_all[:, off + nv_b[b], :],
                    in_=vts[b][T: T + 128, :].rearrange("(o p) d -> p o d", p=128))
                off += nv_b[b] + 1

            psum_pv = pvpool.tile([128, 128], F32, tag="pv")
            nc.vector.memset(psum_pv[:], 0.0)
            for b in range(B):
                chunks = list(range(nv_b[b])) + [NCH - 1]
                for i, c in enumerate(chunks):
                    voff = vbase[b] + (i if i < nv_b[b] else nv_b[b])
                    nc.tensor.matmul(
                        out=psum_pv[32 * b: 32 * b + G, :],
                        lhsT=pT[:, c, 32 * b: 32 * b + G],
                        rhs=v_all[:, voff, :],
                        start=(i == 0),
                        stop=(i == len(chunks) - 1),
                        tile_position=(0, 32 * b),
                    )

            # ---- phase 6: scale by sv and write out ----
            outsb = wpool.tile([128, 128], F32, tag="outsb")
            nc.scalar.mul(out=outsb[:], in_=psum_pv[:], mul=sv)
            for b in range(B):
                nc.sync.dma_start(
                    out=o_out[b].rearrange("(a f) -> a f", a=G),
                    in_=outsb[32 * b: 32 * b + G, :])

    nc.compile()
    return nc


def assemble(results):
    """results: list of 8 dicts with 'o_out' [B, G*D] and 'sel_out' [B,G,KP]."""
    out = np.zeros((B, H, D), np.float32)
    sel = np.zeros((B, H, KP), np.int32)
    for h, r in enumerate(results):
        o = np.asarray(r["o_out"], np.float32).reshape(B, G, D)
        s = np.asarray(r["sel_out"], np.int32).reshape(B, G, KP)
        out[:, h * G: (h + 1) * G, :] = o
        sel[:, h * G: (h + 1) * G, :] = s
    return out, sel


# ----------------------------------------------------------------------------
# SPMD runner: compile once per (lengths, scales) signature, run on 8 cores.
# ----------------------------------------------------------------------------
import threading

_CACHE = {}
_CACHE_LOCK = threading.Lock()


def _get_nc(meta):
    key = (tuple(meta["lengths"]), meta["scale_k"], meta["sv"])
    with _CACHE_LOCK:
        nc = _CACHE.get(key)
        if nc is None:
            nc = build_nc(meta)
            _CACHE[key] = nc
        return nc


def _run(inputs, trace=False):
    from concourse import bass_utils
    per_core, meta = host_prep(**inputs)
    nc = _get_nc(meta)
    res = bass_utils.run_bass_kernel_spmd(
        nc, per_core, core_ids=list(range(HKV)), trace=trace)
    out, sel = assemble(res.results)
    return (out, sel), res


def kernel(**inputs):
    inputs = {k: (np.asarray(v) if not np.isscalar(v) else v)
              for k, v in inputs.items()}
    (out, sel), _ = _run(inputs, trace=False)
    return out, sel


# revision 4
# speedup vs baseline: 1.5153x; 1.5153x over previous
"""Builder + host prep for the sparse decoding-attention TRN2 kernel.

Sharding: 8 cores, one KV head per core (tensor parallel over Hkv=8).
Each core computes out[:, h*G:(h+1)*G, :] and sel[:, h*G:(h+1)*G, :]
for its head h; the host concatenates.
"""

import sys as _sys
for _p in ("/opt/trn_rl_repo",):
    if _p not in _sys.path:
        _sys.path.insert(0, _p)


import numpy as np
from ml_dtypes import bfloat16

import concourse.bass as bass
import concourse.bacc as bacc
import concourse.mybir as mybir
from concourse.tile import TileContext

F32 = mybir.dt.float32
BF16 = mybir.dt.bfloat16
I32 = mybir.dt.int32
U32 = mybir.dt.uint32
U16 = mybir.dt.uint16
I16 = mybir.dt.int16

B, H, HKV, D = 4, 32, 8, 128
G = H // HKV          # 4
P, S = 128, 64        # pages, tokens/page
T = P * S             # 8192
KP = 16               # selected pages (15 top + last)
NTILE = 512           # tokens per score matmul tile
NT = T // NTILE       # 16
NEG = -1.0e9
PF = T + 128          # p grid: 8192 cache tokens + current-token slots (pad)
NCH = PF // 128       # 65 transpose/PV chunks
VROWS = T + 128       # v rows per b: tokens + v_hi/v_mid/v_lo x2 + zero pad


def bf16_split3(x32: np.ndarray):
    """Exact 3-term bf16 decomposition of float32 (hi+mid+lo == x bitwise)."""
    x32 = np.asarray(x32, np.float32)
    hi = x32.astype(bfloat16)
    r1 = x32 - hi.astype(np.float32)
    mid = r1.astype(bfloat16)
    r2 = r1 - mid.astype(np.float32)
    lo = r2.astype(bfloat16)
    return hi, mid, lo


def host_prep(q, k, v, kv_scale_quant_orig, k_cache, v_cache,
              lengths_per_sample, timestep):
    """Returns (per_core_inmaps, meta). Uses jax-on-CPU for rope + current
    scores so the math matches the reference bit-for-bit."""
    import jax
    import jax.numpy as jnp

    cpu = jax.devices("cpu")[0]
    pos = float(int(timestep) - 1)
    sm_scale = 1.0 / np.sqrt(D)

    with jax.default_device(cpu):
        def rope_neox(x, p, base=10000.0):
            half = D // 2
            inv_freq = 1.0 / (base ** (jnp.arange(half, dtype=jnp.float32) * (2.0 / D)))
            ang = p * inv_freq
            c, s = jnp.cos(ang), jnp.sin(ang)
            x1, x2 = x[..., :half], x[..., half:]
            return jnp.concatenate([x1 * c - x2 * s, x1 * s + x2 * c], axis=-1)

        qj = jnp.asarray(np.asarray(q, np.float32))
        kj = jnp.asarray(np.asarray(k, np.float32))
        qh = rope_neox(qj, pos).reshape(B, HKV, G, D)
        k_cur = rope_neox(kj, pos)
        cur = jnp.einsum('bhgd,bhd->bhg', qh, k_cur) * sm_scale
        qh = np.asarray(qh)          # [B, HKV, G, D] f32
        cur = np.asarray(cur)        # [B, HKV, G] f32

    kv_scale = np.asarray(kv_scale_quant_orig, np.float32)
    sk, sv = float(kv_scale[0]), float(kv_scale[1])
    scale_k = float(np.float32(sk) * np.float32(sm_scale))

    k_cache = np.asarray(k_cache)
    v_cache = np.asarray(v_cache)
    # K^T per head: [HKV, D, B*T] bf16 (ints 0..127, exact)
    kc_all = np.ascontiguousarray(
        k_cache.transpose(3, 4, 0, 1, 2).astype(bfloat16)).reshape(HKV, D, B * T)
    # V rows per (head, b): [HKV, B, VROWS, D] bf16
    vt_all = np.zeros((HKV, B, VROWS, D), bfloat16)
    vt_all[:, :, :T, :] = v_cache.transpose(3, 0, 1, 2, 4).astype(bfloat16).reshape(
        HKV, B, T, D)
    # the reference appends the RAW current v (no sv dequant); our kernel
    # scales the whole PV output by sv, so store v/sv here (exact 3-term
    # bf16 split of the f32 quotient). Rows T..T+2 pair with p_hi slots,
    # rows T+3..T+5 with p_lo slots.
    v32 = np.asarray(v, np.float32)
    vq = (v32 / np.float32(sv)).astype(np.float32)
    v_hi, v_mid, v_lo = bf16_split3(vq)
    for h in range(HKV):
        for b in range(B):
            for rep in (0, 3):
                vt_all[h, b, T + rep, :] = v_hi[b, h]
                vt_all[h, b, T + rep + 1, :] = v_mid[b, h]
                vt_all[h, b, T + rep + 2, :] = v_lo[b, h]

    lengths = np.maximum(np.asarray(lengths_per_sample, np.int64), 1)
    lengths = np.minimum(lengths, T).astype(np.int64)

    negbuf = np.full((T,), NEG, np.float32)

    per_core = []
    for h in range(HKV):
        hi, mid, lo = bf16_split3(qh[:, h])          # each [B, G, D]
        # q3[d, b*12 + j*4 + g]
        q3 = np.ascontiguousarray(
            np.stack([hi, mid, lo], axis=1)          # [B, 3, G, D]
            .transpose(3, 0, 1, 2)                   # [D, B, 3, G]
            .reshape(D, B * 12))
        curs = np.zeros((128, 1), np.float32)
        for b in range(B):
            for g in range(G):
                curs[32 * b + g, 0] = cur[b, h, g]
        im = {
            "kt": kc_all[h],                         # [128, 32768] bf16
            "q3": q3,                                # [128, 48] bf16
            "curs": curs,                            # [128, 1] f32
            "negbuf": negbuf,                        # [8192] f32
        }
        for b in range(B):
            # interleave: vi[p, c, :] = vt[c*128 + p, :], so one DMA per b
            # loads [128, nchunk, 128] with a long contiguous line/partition
            im[f"vt{b}"] = np.ascontiguousarray(
                vt_all[h, b].reshape(NCH, 128, D).transpose(1, 0, 2))
        per_core.append(im)

    meta = {
        "lengths": [int(x) for x in lengths],
        "scale_k": scale_k,
        "sv": float(np.float32(sv)),
    }
    return per_core, meta


def build_nc(meta):
    """Builds the compiled per-core Bass program (same NEFF for all cores)."""
    lengths = meta["lengths"]
    scale_k = meta["scale_k"]
    sv = meta["sv"]

    nc = bacc.Bacc("TRN2", target_bir_lowering=False, debug=False)

    kt = nc.dram_tensor("kt", [D, B * T], BF16, kind="ExternalInput").ap()
    q3 = nc.dram_tensor("q3", [D, B * 12], BF16, kind="ExternalInput").ap()
    curs = nc.dram_tensor("curs", [128, 1], F32, kind="ExternalInput").ap()
    negbuf = nc.dram_tensor("negbuf", [T], F32, kind="ExternalInput").ap()
    vts = [nc.dram_tensor(f"vt{b}", [128, NCH, D], BF16, kind="ExternalInput").ap()
           for b in range(B)]

    o_out = nc.dram_tensor("o_out", [B, G * D], F32, kind="ExternalOutput").ap()
    sel_out = nc.dram_tensor("sel_out", [B, G, KP], I32, kind="ExternalOutput").ap()

    nt_b = [min(NT, (lengths[b] + NTILE - 1) // NTILE) for b in range(B)]
    nt_max = max(nt_b)

    with TileContext(nc) as tc:
        with (
            tc.tile_pool(name="const", bufs=1) as cpool,
            tc.tile_pool(name="scores", bufs=1) as spool,
            tc.tile_pool(name="kt", bufs=1) as kpool,
            tc.tile_pool(name="psum_s", bufs=1, space="PSUM") as pspool,
            tc.tile_pool(name="psum_pv", bufs=1, space="PSUM") as pvpool,
            tc.tile_pool(name="work", bufs=1) as wpool,
            tc.tile_pool(name="vsel", bufs=2) as vpool,
        ):
            q3_sb = cpool.tile([D, B * 12], BF16, tag="q3")
            nc.sync.dma_start(out=q3_sb[:], in_=q3)

            scores_sb = spool.tile([128, T], F32, tag="scores")

            # Persistent rotating PSUM tiles, zeroed once: partitions outside
            # the 4 matmul column groups stay 0.0 so no NaN garbage reaches
            # the stats/top-k path.
            NPS = 4
            psum_tiles = [pspool.tile([128, NTILE], F32, tag=f"ps{i}",
                                      name=f"ps{i}")
                          for i in range(NPS)]
            for pt in psum_tiles:
                nc.vector.memset(pt[:], 0.0)

            # ---- phase 1: scores = (q . k) * scale_k for all tokens ----
            # one big K DMA per batch; matmuls slice the resident tile
            ktiles = []
            for b in range(B):
                ktile = kpool.tile([D, nt_b[b] * NTILE], BF16, tag=f"kt{b}",
                                   name=f"kt{b}")
                nc.sync.dma_start(
                    out=ktile[:],
                    in_=kt[:, b * T: b * T + nt_b[b] * NTILE])
                ktiles.append(ktile)
            for t in range(nt_max):
                psum = psum_tiles[t % NPS]
                for b in range(B):
                    if t >= nt_b[b]:
                        continue
                    for j in range(3):
                        nc.tensor.matmul(
                            out=psum[32 * b: 32 * b + G, :],
                            lhsT=q3_sb[:, b * 12 + j * 4: b * 12 + j * 4 + 4],
                            rhs=ktiles[b][:, t * NTILE: (t + 1) * NTILE],
                            start=(j == 0),
                            stop=(j == 2),
                            tile_position=(0, 32 * b),
                        )
                nc.scalar.mul(
                    out=scores_sb[:, t * NTILE: (t + 1) * NTILE],
                    in_=psum[:], mul=scale_k)

            # columns past the longest computed tile are never written by the
            # PSUM copies; zero them so the stats reduce reads defined data
            if nt_max < NT:
                nc.vector.memset(scores_sb[:, nt_max * NTILE: T], 0.0)

            # mask the invalid tail [L_b, T) with NEG via broadcast DMA
            for b in range(B):
                L = lengths[b]
                if L < T:
                    nc.sync.dma_start(
                        out=scores_sb[32 * b: 32 * b + G, L:T],
                        in_=negbuf[0: T - L].unsqueeze(0).to_broadcast([G, T - L]))

            # ---- phase 2: page stats + top-k ----
            stats = wpool.tile([128, P], F32, tag="stats")
            nc.vector.reduce_max(
                out=stats[:],
                in_=scores_sb[:].rearrange("p (a b) -> p a b", b=S),
                axis=mybir.AxisListType.X)

            mx_v = wpool.tile([128, 8], F32, tag="mx_v")
            mx_i = wpool.tile([128, 8], U32, tag="mx_i")
            stats2 = wpool.tile([128, P - 1], F32, tag="stats2")
            mx2_v = wpool.tile([128, 8], F32, tag="mx2_v")
            mx2_i = wpool.tile([128, 8], U32, tag="mx2_i")

            nc.vector.max(out=mx_v[:], in_=stats[:, 0: P - 1])
            nc.vector.max_index(out=mx_i[:], in_max=mx_v[:], in_values=stats[:, 0: P - 1])
            nc.vector.match_replace(
                out=stats2[:], in_to_replace=mx_v[:], in_values=stats[:, 0: P - 1],
                imm_value=-1.0e30)
            nc.vector.max(out=mx2_v[:], in_=stats2[:])
            nc.vector.max_index(out=mx2_i[:], in_max=mx2_v[:], in_values=stats2[:])

            sel_i32 = wpool.tile([128, KP], I32, tag="sel_i32")
            nc.vector.tensor_copy(out=sel_i32[:, 0:8], in_=mx_i[:])
            nc.vector.tensor_copy(out=sel_i32[:, 8:15], in_=mx2_i[:, 0:7])
            nc.vector.memset(sel_i32[:, 15:16], P - 1)
            for b in range(B):
                nc.sync.dma_start(out=sel_out[b], in_=sel_i32[32 * b: 32 * b + G, :])

            # ---- phase 3: selection mask + masked softmax (unnormalized) ----
            # selection set == {stats >= 15th-largest} + forced last page
            thresh = mx2_v[:, 6:7]
            mask01 = wpool.tile([128, P], F32, tag="mask01")
            nc.vector.tensor_scalar(
                out=mask01[:], in0=stats[:], scalar1=thresh, scalar2=None,
                op0=mybir.AluOpType.is_ge)
            nc.vector.memset(mask01[:, P - 1: P], 1.0)
            tmask = wpool.tile([128, P], F32, tag="tmask")
            nc.vector.tensor_scalar(
                out=tmask[:], in0=mask01[:], scalar1=1.0, scalar2=-NEG,
                op0=mybir.AluOpType.subtract, op1=mybir.AluOpType.mult)

            curs_sb = wpool.tile([128, 1], F32, tag="curs_sb")
            nc.sync.dma_start(out=curs_sb[:], in_=curs)
            # row max over the selected set comes from the page stats
            mstat = wpool.tile([128, P], F32, tag="mstat")
            nc.vector.tensor_tensor(out=mstat[:], in0=stats[:], in1=tmask[:],
                                    op=mybir.AluOpType.add)
            m0 = wpool.tile([128, 1], F32, tag="m0")
            m = wpool.tile([128, 1], F32, tag="m")
            negm = wpool.tile([128, 1], F32, tag="negm")
            nc.vector.reduce_max(out=m0[:], in_=mstat[:], axis=mybir.AxisListType.X)
            nc.vector.tensor_tensor(out=m[:], in0=m0[:], in1=curs_sb[:],
                                    op=mybir.AluOpType.max)
            nc.vector.tensor_scalar_mul(negm[:], m[:], -1.0)

            # chunked mask-add + exp straight to bf16 p (unnormalized), with
            # per-chunk partial sums; the 1/sum lands in the output scale.
            p_full = wpool.tile([128, PF], BF16, tag="p_full")
            sums = wpool.tile([128, NT + 1], F32, tag="sums")
            ecur = wpool.tile([128, 1], F32, tag="ecur")
            for t in range(NT):
                sl = scores_sb[:, t * NTILE: (t + 1) * NTILE]
                nc.vector.scalar_tensor_tensor(
                    out=sl.rearrange("p (a b) -> p a b", b=S),
                    in0=tmask[:, t * 8: (t + 1) * 8].unsqueeze(2).to_broadcast(
                        [128, 8, S]),
                    scalar=0.0,
                    in1=sl.rearrange("p (a b) -> p a b", b=S),
                    op0=mybir.AluOpType.add,
                    op1=mybir.AluOpType.add)
                nc.scalar.activation(
                    out=p_full[:, t * NTILE: (t + 1) * NTILE], in_=sl,
                    func=mybir.ActivationFunctionType.Exp,
                    bias=negm[:], scale=1.0, accum_out=sums[:, t: t + 1])
            nc.scalar.activation(
                out=ecur[:], in_=curs_sb[:],
                func=mybir.ActivationFunctionType.Exp,
                bias=negm[:], scale=1.0)
            nc.vector.tensor_copy(out=sums[:, NT: NT + 1], in_=ecur[:])
            sumall = wpool.tile([128, 1], F32, tag="sumall")
            rec = wpool.tile([128, 1], F32, tag="rec")
            rs = wpool.tile([128, 1], F32, tag="rs")
            nc.vector.reduce_sum(out=sumall[:], in_=sums[:], axis=mybir.AxisListType.X)
            nc.vector.reciprocal(out=rec[:], in_=sumall[:])
            nc.vector.tensor_scalar_mul(rs[:], rec[:], sv)

            # current-token slots: unnormalized p_cur = ecur, exact hi+lo
            phi_f = wpool.tile([128, 1], F32, tag="phi_f")
            plo_f = wpool.tile([128, 1], F32, tag="plo_f")
            nc.vector.tensor_copy(out=p_full[:, T: T + 1], in_=ecur[:])
            nc.vector.tensor_copy(out=phi_f[:], in_=p_full[:, T: T + 1])
            nc.vector.tensor_tensor(out=plo_f[:], in0=ecur[:], in1=phi_f[:],
                                    op=mybir.AluOpType.subtract)
            nc.vector.tensor_copy(out=p_full[:, T + 1: T + 2], in_=p_full[:, T: T + 1])
            nc.vector.tensor_copy(out=p_full[:, T + 2: T + 3], in_=p_full[:, T: T + 1])
            nc.vector.tensor_copy(out=p_full[:, T + 3: T + 4], in_=plo_f[:])
            nc.vector.tensor_copy(out=p_full[:, T + 4: T + 5], in_=p_full[:, T + 3: T + 4])
            nc.vector.tensor_copy(out=p_full[:, T + 5: T + 6], in_=p_full[:, T + 3: T + 4])
            nc.vector.memset(p_full[:, T + 6: PF], 0.0)

            # ---- phase 4: transpose p into [token, row] chunks ----
            pT = wpool.tile([128, NCH, 128], BF16, tag="pT")
            nv_b = [min(64, (lengths[b] + 127) // 128) for b in range(B)]
            nv_max = max(nv_b)
            teng = [nc.sync, nc.scalar]
            tn = 0
            for c in list(range(nv_max)) + [NCH - 1]:
                teng[tn % 2].dma_start_transpose(
                    out=pT[:, c, :], in_=p_full[:, c * 128: (c + 1) * 128])
                tn += 1

            # ---- phase 5: dense PV, V prefetched per batch (2 bufs) ----
            psum_pv = pvpool.tile([128, 128], F32, tag="pv")
            nc.vector.memset(psum_pv[:], 0.0)
            for b in range(B):
                vb = vpool.tile([128, nv_max + 1, 128], BF16, tag="vb")
                nc.sync.dma_start(
                    out=vb[:, 0: nv_b[b], :], in_=vts[b][:, 0: nv_b[b], :])
                nc.sync.dma_start(
                    out=vb[:, nv_b[b], :], in_=vts[b][:, NCH - 1, :])
                for i, c in enumerate(list(range(nv_b[b])) + [NCH - 1]):
                    voff = i
                    nc.tensor.matmul(
                        out=psum_pv[32 * b: 32 * b + G, :],
                        lhsT=pT[:, c, 32 * b: 32 * b + G],
                        rhs=vb[:, voff, :],
                        start=(i == 0),
                        stop=(i == nv_b[b]),
                        tile_position=(0, 32 * b),
                    )

            # ---- phase 6: scale by rec*sv and write out ----
            outsb = wpool.tile([128, 128], F32, tag="outsb")
            nc.scalar.activation(
                out=outsb[:], in_=psum_pv[:],
                func=mybir.ActivationFunctionType.Copy, scale=rs[:])
            for b in range(B):
                nc.sync.dma_start(
                    out=o_out[b].rearrange("(a f) -> a f", a=G),
                    in_=outsb[32 * b: 32 * b + G, :])

    nc.compile()
    return nc


def assemble(results):
    """results: list of 8 dicts with 'o_out' [B, G*D] and 'sel_out' [B,G,KP]."""
    out = np.zeros((B, H, D), np.float32)
    sel = np.zeros((B, H, KP), np.int32)
    for h, r in enumerate(results):
        o = np.asarray(r["o_out"], np.float32).reshape(B, G, D)
        s = np.asarray(r["sel_out"], np.int32).reshape(B, G, KP)
        out[:, h * G: (h + 1) * G, :] = o
        sel[:, h * G: (h + 1) * G, :] = s
    return out, sel


# ----------------------------------------------------------------------------
# SPMD runner: compile once per (lengths, scales) signature, run on 8 cores.
# ----------------------------------------------------------------------------
import threading

_CACHE = {}
_CACHE_LOCK = threading.Lock()


def _get_nc(meta):
    key = (tuple(meta["lengths"]), meta["scale_k"], meta["sv"])
    with _CACHE_LOCK:
        nc = _CACHE.get(key)
        if nc is None:
            nc = build_nc(meta)
            _CACHE[key] = nc
        return nc


def _run(inputs, trace=False):
    from concourse import bass_utils
    per_core, meta = host_prep(**inputs)
    nc = _get_nc(meta)
    res = bass_utils.run_bass_kernel_spmd(
        nc, per_core, core_ids=list(range(HKV)), trace=trace)
    out, sel = assemble(res.results)
    return (out, sel), res


def kernel(**inputs):
    inputs = {k: (np.asarray(v) if not np.isscalar(v) else v)
              for k, v in inputs.items()}
    (out, sel), _ = _run(inputs, trace=False)
    return out, sel
